# revision 29
# baseline (speedup 1.0000x reference)
"""Trainium2 Bass kernel for nn_CFM_80272938762374 (dense_mlp).

Reference computation (T=1024, O=512, D=256, H=512):
    ht = z_t @ W1[:D]                  # [T, H]
    ho = z_o @ W1[D:]                  # [O, H]
    h  = leaky_relu(ht[:,None,:] + ho[None,:,:] + b1, 0.01)   # [T, O, H]
    out = squeeze(h @ W2, -1) + b2[0]  # [T, O]

Strategy (8 cores, O sharded 64-wide per core; all FLOPs on device; host
does only layout prep - transposes, slicing, weight scaling/casts):

    leaky_relu(x) = 0.99*relu(x) + 0.01*x, so with g = ho + b1:
      out[t,o] = sum_k 0.99*W2[k]*relu(htT[k,t] + g[k,o])
               + 0.01*(sum_k W2[k]*htT[k,t])        # ct[t], o-independent
               + (0.01*sum_k W2[k]*g[k,o] + b2)     # co[o], t-independent

    Per core:
    * PE computes htT (fp16 matmuls, 1 cyc/row) and g (fp32, exact) once;
      ct lands replicated across 128 psum rows via column-replicated 0.01*W2
      weights and is copied to SBUF; co+b2 is produced as a [1,64] row and
      scattered to co_arr[32j, og] for use as a per-partition bias.
    * Main loop (64 o's x 4 k-blocks): ONE fused op produces each relu tile
      [128k x 1024t] in fp16 - DVE tensor_scalar(add per-partition g-col,
      max 0) at 4x mode, or ACT Relu-with-bias - and PE contracts it with
      0.99*W2[kblock] ([128,1] fp16 weights, N=512 per PSUM bank). The M=1
      output rows pack 4 o's per psum tile at partitions {0,32,64,96} via
      tile_position col-groups.
    * Drain: one DVE scalar_tensor_tensor per (group, t-half) computes
      (psum + co_col) + ct_rows in a single pass; a strided DMA ships rows
      {0,32,64,96} straight to DRAM. Host concatenates the per-core [64,1024]
      slabs and transposes.

    Modeled (CoreSim cost model): ~125 us/core; PE busy ~116 us of which
    ~109 us is the irreducible relu-volume stream (T*O*H/8 elements at
    128 lanes * 2.4 GHz). Measured rel err vs fp32 reference: ~4e-4.
"""

import os

os.environ.setdefault("JAX_PLATFORMS", "axon")

import numpy as np

import concourse.bacc as bacc
import concourse.tile as tile
from concourse import mybir
from concourse.bass_utils import run_bass_kernel_spmd

F32 = mybir.dt.float32
FP16 = mybir.dt.float16
AOP = mybir.AluOpType
AF = mybir.ActivationFunctionType

T, O, D, H = 1024, 512, 256, 512
NCORES = 8
OL = O // NCORES          # 64 o's per core
KB = H // 128             # 4 k-blocks
DC = D // 128             # 2 d-chunks
TH = 2                    # two 512-wide t halves (PSUM bank limit)
NT = T // TH              # 512
OG = OL // 4              # 16 groups of 4 o's

_cache = {}


def _build():
    nc = bacc.Bacc(
        "TRN2", target_bir_lowering=False, debug=False, num_devices=NCORES
    )

    zt_T = nc.dram_tensor("zt_T", [D, T], FP16, kind="ExternalInput").ap()
    zo_T = nc.dram_tensor("zo_T", [D, OL], F32, kind="ExternalInput").ap()
    w1a = nc.dram_tensor("w1a", [D, H], FP16, kind="ExternalInput").ap()
    w1b = nc.dram_tensor("w1b", [D, H], F32, kind="ExternalInput").ap()
    w2p99 = nc.dram_tensor("w2p99", [H, 1], FP16, kind="ExternalInput").ap()
    w2p01 = nc.dram_tensor("w2p01", [H, 1], F32, kind="ExternalInput").ap()
    w2p01r = nc.dram_tensor("w2p01r", [H, 128], FP16, kind="ExternalInput").ap()
    b1c = nc.dram_tensor("b1c", [H, 1], F32, kind="ExternalInput").ap()
    b2m = nc.dram_tensor("b2m", [1, 1], F32, kind="ExternalInput").ap()
    out_d = nc.dram_tensor("out", [OL, T], F32, kind="ExternalOutput").ap()

    with tile.TileContext(nc) as tc:
        with (
            tc.tile_pool(name="const", bufs=1) as cpool,
            tc.tile_pool(name="rpool", bufs=6) as rpool,
            tc.tile_pool(name="spool", bufs=4) as spool,
            tc.psum_pool(name="ps_hold", bufs=1) as ps_hold,
        ):
            # ---- load constants/weights ----
            def load(name, src, shape, dt=F32, eng=None):
                t = cpool.tile(shape, dt, name=name, tag=name)
                (eng or nc.sync).dma_start(out=t[:], in_=src)
                return t

            zt_sb = [
                load(f"zt{dc}", zt_T[dc * 128:(dc + 1) * 128, :], [128, T],
                     FP16)
                for dc in range(DC)
            ]
            w1a_sb = [
                load(f"w1a{dc}", w1a[dc * 128:(dc + 1) * 128, :], [128, H],
                     FP16, nc.gpsimd)
                for dc in range(DC)
            ]
            zo_sb = [
                load(f"zo{dc}", zo_T[dc * 128:(dc + 1) * 128, :], [128, OL],
                     F32, nc.gpsimd)
                for dc in range(DC)
            ]
            w1b_sb = [
                load(f"w1b{dc}", w1b[dc * 128:(dc + 1) * 128, :], [128, H],
                     F32, nc.gpsimd)
                for dc in range(DC)
            ]
            w99_sb = [
                load(f"w99_{kb}", w2p99[kb * 128:(kb + 1) * 128, :], [128, 1], FP16)
                for kb in range(KB)
            ]
            w01_sb = [
                load(f"w01_{kb}", w2p01[kb * 128:(kb + 1) * 128, :], [128, 1])
                for kb in range(KB)
            ]
            w01r_sb = [
                load(f"w01r{kb}", w2p01r[kb * 128:(kb + 1) * 128, :], [128, 128],
                     FP16)
                for kb in range(KB)
            ]
            b1_sb = [
                load(f"b1_{kb}", b1c[kb * 128:(kb + 1) * 128, :], [128, 1])
                for kb in range(KB)
            ]
            b2_sb = load("b2s", b2m[:, :], [1, 1])
            ones64 = cpool.tile([1, 64], F32, name="ones64", tag="ones64")
            nc.vector.memset(ones64[:], 1.0)

            # ---- setup: htT[k,t] = W1a.T @ z_t.T  (fp32, exact) ----
            htT = [
                cpool.tile([128, T], FP16, name=f"htT{kb}", tag=f"htT{kb}")
                for kb in range(KB)
            ]
            with tc.psum_pool(name="ps_setup", bufs=2) as ps_setup:
                for kb in range(KB):
                    ks = slice(kb * 128, (kb + 1) * 128)
                    for th in range(TH):
                        ts = slice(th * NT, (th + 1) * NT)
                        pht = ps_setup.tile(
                            [128, NT], F32, name="pht", tag="pht"
                        )
                        for dc in range(DC):
                            nc.tensor.matmul(
                                pht[:],
                                lhsT=w1a_sb[dc][:, ks],
                                rhs=zt_sb[dc][:, ts],
                                start=(dc == 0),
                                stop=(dc == DC - 1),
                            )
                        nc.scalar.activation(htT[kb][:, ts], pht[:], AF.Copy)

                # ---- setup: g[k,o] = W1b.T @ z_o.T + b1 ----
                g_sb = [
                    cpool.tile([128, OL], F32, name=f"g{kb}", tag=f"g{kb}")
                    for kb in range(KB)
                ]
                for kb in range(KB):
                    ks = slice(kb * 128, (kb + 1) * 128)
                    pg = ps_setup.tile([128, OL], F32, name="pg", tag="pg")
                    for dc in range(DC):
                        nc.tensor.matmul(
                            pg[:],
                            lhsT=w1b_sb[dc][:, ks],
                            rhs=zo_sb[dc][:],
                            start=(dc == 0),
                            stop=(dc == DC - 1),
                        )
                    nc.scalar.activation(
                        g_sb[kb][:], pg[:], AF.Identity, bias=b1_sb[kb][:, 0:1]
                    )


                # ---- ct, replicated to all 128 rows, landed in SBUF ----
                pct_sb = []
                for th in range(TH):
                    ts = slice(th * NT, (th + 1) * NT)
                    p = ps_hold.tile(
                        [128, NT], F32, name=f"pct{th}", tag=f"pct{th}"
                    )
                    for kb in range(KB):
                        nc.tensor.matmul(
                            p[:],
                            lhsT=w01r_sb[kb][:],
                            rhs=htT[kb][:, ts],
                            start=(kb == 0),
                            stop=(kb == KB - 1),
                        )
                    c = cpool.tile(
                        [128, NT], F32, name=f"ctsb{th}", tag=f"ctsb{th}"
                    )
                    nc.scalar.activation(c[:], p[:], AF.Copy)
                    pct_sb.append(c)

                # ---- co bias row: co[o] = sum_k 0.01*W2[k]*g[k,o] + b2,
                # produced as a [1, 64] row then scattered to co_arr[32j, og]
                # (j = o % 4, og = o // 4) as a per-partition drain bias.
                pco = ps_hold.tile([1, OL], F32, name="pco", tag="pco")
                for kb in range(KB):
                    nc.tensor.matmul(
                        pco[:],
                        lhsT=w01_sb[kb][:],
                        rhs=g_sb[kb][:],
                        start=(kb == 0),
                        stop=False,
                    )
                nc.tensor.matmul(
                    pco[:],
                    lhsT=b2_sb[:],
                    rhs=ones64[:],
                    start=False,
                    stop=True,
                )
                co_row = cpool.tile([1, OL], F32, name="co_row", tag="co_row")
                nc.scalar.activation(co_row[:], pco[:], AF.Copy)
                co_arr = cpool.tile([128, OG], F32, name="co_arr", tag="co_arr")
                for j in range(4):
                    nc.sync.dma_start(
                        out=co_arr[32 * j:32 * j + 1, :],
                        in_=co_row[0:1, :].rearrange(
                            "p (g j) -> p j g", j=4
                        )[:, j, :],
                    )

            # ---- main loop ----
            ps_g_ctx = tc.psum_pool(name="ps_g", bufs=2)
            ps_g = ps_g_ctx.__enter__()
            prod = 0
            for og in range(OG):
                pgrp = [
                    ps_g.tile([128, NT], F32, name=f"pgrp{th}", tag=f"pgrp{th}")
                    for th in range(TH)
                ]
                for j in range(4):
                    o = og * 4 + j
                    for kb in range(KB):
                        r = rpool.tile([128, T], FP16, name="r", tag="r")
                        gcol = g_sb[kb][:, o:o + 1]
                        if prod % 4 != 3:
                            nc.vector.tensor_scalar(
                                out=r[:], in0=htT[kb][:], scalar1=gcol,
                                scalar2=0.0, op0=AOP.add, op1=AOP.max,
                            )
                        else:
                            nc.scalar.activation(
                                r[:], htT[kb][:], AF.Relu, bias=gcol
                            )
                        prod += 1
                        for th in range(TH):
                            ts = slice(th * NT, (th + 1) * NT)
                            nc.tensor.matmul(
                                pgrp[th][32 * j:32 * j + 1, :],
                                lhsT=w99_sb[kb][:],
                                rhs=r[:, ts],
                                start=(kb == 0),
                                stop=(kb == KB - 1),
                                tile_position=(0, 32 * j),
                                skip_group_check=True,
                            )
                # drain with fused co+b2 bias (per-partition), then add the
                # ct rows, then strided DMA of rows {0,32,64,96} to DRAM.
                for th in range(TH):
                    fin = spool.tile([128, NT], F32, name="fin", tag="fin")
                    nc.vector.scalar_tensor_tensor(
                        out=fin[:], in0=pgrp[th][:],
                        scalar=co_arr[:, og:og + 1], in1=pct_sb[th][:],
                        op0=AOP.add, op1=AOP.add,
                    )
                    rows = fin.rearrange("(a b) f -> a b f", b=32)[:, 0, :]
                    nc.sync.dma_start(
                        out=out_d[og * 4:(og + 1) * 4,
                                  th * NT:(th + 1) * NT],
                        in_=rows,
                    )

            ps_g_ctx.__exit__(None, None, None)

    nc.compile()
    return nc


def _get_nc():
    if "nc" not in _cache:
        _cache["nc"] = _build()
    return _cache["nc"]


def kernel(z_t, z_o, W1, b1, W2, b2, **run_kwargs):
    z_t = np.asarray(z_t, np.float32)
    z_o = np.asarray(z_o, np.float32)
    W1 = np.asarray(W1, np.float32)
    b1 = np.asarray(b1, np.float32)
    W2 = np.asarray(W2, np.float32)
    b2 = np.asarray(b2, np.float32)

    nc = _get_nc()

    zt_T = np.ascontiguousarray(z_t.T.astype(np.float16))   # [D, T]
    w1a = np.ascontiguousarray(W1[:D].astype(np.float16))   # [D, H]
    w1b = np.ascontiguousarray(W1[D:])                      # [D, H]
    w2p99 = np.ascontiguousarray((0.99 * W2).astype(np.float16))
    w2p01 = np.ascontiguousarray(0.01 * W2)                 # [H, 1]
    w2p01r = np.ascontiguousarray(
        np.repeat((0.01 * W2).astype(np.float16), 128, 1))
    b1c = np.ascontiguousarray(b1.reshape(H, 1))
    b2m = np.ascontiguousarray(b2.reshape(1, 1))

    in_maps = []
    for c in range(NCORES):
        zo_T = np.ascontiguousarray(z_o[c * OL:(c + 1) * OL].T)  # [D, OL]
        in_maps.append({
            "zt_T": zt_T, "zo_T": zo_T, "w1a": w1a, "w1b": w1b,
            "w2p99": w2p99, "w2p01": w2p01, "w2p01r": w2p01r,
            "b1c": b1c, "b2m": b2m,
        })

    res = run_bass_kernel_spmd(
        nc, in_maps, core_ids=list(range(NCORES)), **run_kwargs
    )
    out_T = np.concatenate(
        [res.results[c]["out"] for c in range(NCORES)], axis=0
    )  # [O, T]
    if run_kwargs:
        _cache["last_results"] = res
    return np.ascontiguousarray(out_T.T).astype(np.float32)

